# revision 31
# baseline (speedup 1.0000x reference)
"""AttentionBlock (GroupNorm -> 1x1-conv QKV -> softmax attention -> 1x1-conv proj
-> residual) for Trainium2, data-parallel over batch across 8 NeuronCores.

Shapes (hardcoded): x [B=8, C=64, H=64, W=64] fp32; N = H*W = 4096.
Each core processes one sample end-to-end; no cross-core communication.

Key Trainium facts that shape this kernel:
  - A matmul with contraction K<=64 streams at HALF rate (64-row tiling mode);
    K=128 streams 1 column/cycle. With C=64 channels, all hot matmuls are
    made K=128 by duplicating operands on both partition halves and halving
    the stacked weights (sum over 128 partitions of duplicated data = 2x).
  - fp32 matmuls run as two PE passes and their self-loading LDWEIGHTS only
    supports one sync wait; bf16 is one pass (and scores are O(1), so bf16
    keeps ~3 digits -> final error ~1e-4).
  - ScalarE exp runs at 1 elem/lane/cycle -> 16.7M exps/core ~ 115us is the
    roofline engine; everything else is arranged to hide under it.

Per-core pipeline:
  1. GroupNorm: per-channel bn_stats/bn_aggr on x2x[0:64] -> tiny mask
     matmuls reduce/broadcast the 8-channel groups -> one fused affine
     produces h2x [128, N] bf16 (h duplicated on both partition halves).
  2. q2x = (Wq h + bq)/16 and k2x = Wk h, both [128, N] bf16 duplicated
     (bk dropped: constant shift per softmax row). vT [N, C+1] bf16 with a
     ones column so the AV matmul also accumulates the softmax denominator.
  3. sT[m, n] tiles = k2x.T @ q2x (K=128), exp on ScalarE PSUM->SBUF (score
     range is ~[-3, 3]: no row-max subtraction needed), AV accumulates
     out[c, n] + den[n] over the 32 m-chunks.
  4. proj = Wp @ out_unnormalized, scaled by 1/den (column scaling commutes
     with the left matmul; reciprocal via a DMA partition-broadcast of den
     and the fast DVE approx reciprocal), + (bp + Wp bv) + residual x.

The nt loop is software-pipelined (scores/exp of tile nt interleaved with AV
of tile nt-1) so the PE stream stays dense and ScalarE never starves.
"""

import os
import numpy as np
import ml_dtypes

import concourse.bass as bass
import concourse.bacc as bacc
import concourse.mybir as mybir
from concourse.tile import TileContext
from concourse.bass_utils import run_bass_kernel_spmd

FP = mybir.dt.float32
F16 = mybir.dt.bfloat16
B, C, H, W = 8, 64, 64, 64
N = H * W          # 4096
G = 8              # groups
NT = 512           # n-tile (free dim of score tiles)
MT = 128           # m-tile (partition dim of score tiles)
N_NT = N // NT     # 8
N_MT = N // MT     # 32
NPAIR = N_MT // 2  # 16 score psum groups (2 m-chunks each) per n-tile
EPS = 1e-5
COPY = mybir.ActivationFunctionType.Copy

last_run_info = {}


def build_program(debug=False):
    # Bacc (not raw Bass): its finalize pipeline splits multi-sem waits.
    nc = bacc.Bacc()
    dbg = {}
    if debug:
        for nm, shp in [("dbg_h", [128, N]), ("dbg_q", [128, N]), ("dbg_k", [128, N]),
                        ("dbg_vt", [128, N_MT * (C + 1)]),
                        ("dbg_av", [C, N]), ("dbg_den", [1, N])]:
            dbg[nm] = nc.dram_tensor(nm, shp, FP, kind="ExternalOutput")

    x_d = nc.dram_tensor("x", [C, N], FP, kind="ExternalInput")
    wq_st_d = nc.dram_tensor("wq_st", [128, 128], F16, kind="ExternalInput")  # tile(Wq.T,(2,2))/32
    wk_st_d = nc.dram_tensor("wk_st", [128, 128], F16, kind="ExternalInput")  # tile(Wk.T,(2,2))/2
    wv_st_d = nc.dram_tensor("wv_st", [128, C], F16, kind="ExternalInput")    # tile(Wv.T,(2,1))/2
    bq2_d = nc.dram_tensor("bq2", [128, 1], FP, kind="ExternalInput")         # tile(bq,2)/16
    wpT_d = nc.dram_tensor("wpT", [C, C], F16, kind="ExternalInput")          # Wp.T
    bpp_d = nc.dram_tensor("bpp", [C, 1], FP, kind="ExternalInput")           # bp + Wp@bv
    gamma2_d = nc.dram_tensor("gamma2", [128, 1], FP, kind="ExternalInput")   # tile(gn_w,2)
    beta2_d = nc.dram_tensor("beta2", [128, 1], FP, kind="ExternalInput")     # tile(gn_b,2)
    gmask_d = nc.dram_tensor("gmask", [C, G], FP, kind="ExternalInput")       # 1/8 blocks
    gbcast2_d = nc.dram_tensor("gbcast2", [G, 128], FP, kind="ExternalInput") # 1 blocks x2
    out_d = nc.dram_tensor("out", [C, N], FP, kind="ExternalOutput")

    with TileContext(nc) as tc:
        with (
            tc.tile_pool(name="const", bufs=1) as const,
            tc.tile_pool(name="big", bufs=1) as big,
            tc.tile_pool(name="epool", bufs=2) as epool,
            tc.tile_pool(name="small", bufs=4) as small,
            tc.tile_pool(name="outp", bufs=3) as outp,
            tc.tile_pool(name="dram", bufs=2, space="DRAM") as drampool,
            tc.tile_pool(name="qk_ps", bufs=2, space="PSUM") as qk_ps,
            tc.tile_pool(name="av_ps", bufs=1, space="PSUM") as av_ps,
            tc.tile_pool(name="post_ps", bufs=1, space="PSUM") as post_ps,
        ):
            # ---- constant loads ----
            bq2 = const.tile([128, 1], FP, tag="bq2")
            bpp = const.tile([C, 1], FP, tag="bpp")
            gamma2 = const.tile([128, 1], FP, tag="gamma2")
            beta2 = const.tile([128, 1], FP, tag="beta2")
            for t, d in [(bq2, bq2_d), (bpp, bpp_d), (gamma2, gamma2_d), (beta2, beta2_d)]:
                nc.sync.dma_start(out=t[:], in_=d[:])
            # Matmul operands straight off DMA would need DMA+DVE waits, but
            # a matmul's LDWEIGHTS supports only one sync wait. Funnel
            # weights through a DVE copy so matmul deps collapse onto DVE.
            wq_st = const.tile([128, 128], F16, tag="wq_st")
            wk_st = const.tile([128, 128], F16, tag="wk_st")
            wv_st = const.tile([128, C], F16, tag="wv_st")
            wpT = const.tile([C, C], F16, tag="wpT")
            gmask = const.tile([C, G], FP, tag="gmask")
            gbcast2 = const.tile([G, 128], FP, tag="gbcast2")
            for t, d in [(wq_st, wq_st_d), (wk_st, wk_st_d), (wv_st, wv_st_d),
                         (wpT, wpT_d), (gmask, gmask_d), (gbcast2, gbcast2_d)]:
                stg = small.tile(list(t.shape), t.dtype, tag=f"stage_{t.shape[1]}_{t.dtype}")
                nc.sync.dma_start(out=stg[:], in_=d[:])
                nc.vector.tensor_copy(out=t[:], in_=stg[:])

            eps_sb = const.tile([128, 1], FP, tag="eps")
            nc.vector.memset(eps_sb[:], EPS)

            # ---- load x duplicated on both partition halves ----
            x2x = big.tile([128, N], FP, tag="x2x")
            for j in range(4):
                sl = slice(j * (N // 4), (j + 1) * (N // 4))
                nc.sync.dma_start(out=x2x[0:C, sl], in_=x_d[:, sl])
                nc.sync.dma_start(out=x2x[C:128, sl], in_=x_d[:, sl])

            # ---- GroupNorm ----
            stats = small.tile([C, N // 512, 6], FP, tag="gn_stats")
            for j in range(N // 512):
                nc.vector.bn_stats(out=stats[:, j, :], in_=x2x[0:C, j * 512:(j + 1) * 512])
            mv = small.tile([C, 2], FP, tag="gn_mv")
            nc.vector.bn_aggr(out=mv[:], in_=stats[:])
            # mm2 = [mean_c, mean_c^2 + var_c]
            mm2 = small.tile([C, 2], FP, tag="gn_mm2")
            nc.vector.tensor_copy(out=mm2[:, 0:1], in_=mv[:, 0:1])
            t0 = small.tile([C, 1], FP, tag="gn_t0")
            nc.vector.tensor_mul(out=t0[:], in0=mv[:, 0:1], in1=mv[:, 0:1])
            nc.vector.tensor_add(out=mm2[:, 1:2], in0=t0[:], in1=mv[:, 1:2])
            # group stats: [G, 2] = gmask.T @ mm2   (gmask holds 1/8)
            gstat_ps = post_ps.tile([128, 512], FP, tag="post")
            nc.tensor.matmul(out=gstat_ps[0:G, 0:2], lhsT=gmask[:], rhs=mm2[:])
            gstat = small.tile([G, 2], FP, tag="gn_gstat")
            nc.vector.tensor_copy(out=gstat[:], in_=gstat_ps[0:G, 0:2])
            # var_g = E[x^2]_g - mean_g^2 ; rstd = 1/sqrt(var+eps)
            vg = small.tile([G, 1], FP, tag="gn_vg")
            nc.vector.tensor_mul(out=vg[:], in0=gstat[:, 0:1], in1=gstat[:, 0:1])
            nc.vector.tensor_sub(out=vg[:], in0=gstat[:, 1:2], in1=vg[:])
            stdg = small.tile([G, 1], FP, tag="gn_stdg")
            nc.scalar.activation(out=stdg[:], in_=vg[:],
                                 func=mybir.ActivationFunctionType.Sqrt,
                                 bias=eps_sb[0:G, :])
            rhs2 = small.tile([G, 2], FP, tag="gn_rhs2")
            nc.vector.tensor_copy(out=rhs2[:, 0:1], in_=gstat[:, 0:1])
            nc.vector.reciprocal(out=rhs2[:, 1:2], in_=stdg[:])
            # broadcast to both channel copies: [128, 2] = gbcast2.T @ rhs2
            pstat_ps = post_ps.tile([128, 512], FP, tag="post")
            nc.tensor.matmul(out=pstat_ps[:, 0:2], lhsT=gbcast2[:], rhs=rhs2[:])
            a_sb = small.tile([128, 1], FP, tag="gn_a")
            b_sb = small.tile([128, 1], FP, tag="gn_b")
            nc.vector.tensor_mul(out=a_sb[:], in0=pstat_ps[:, 1:2], in1=gamma2[:])
            nc.vector.tensor_mul(out=b_sb[:], in0=pstat_ps[:, 0:1], in1=a_sb[:])
            nc.vector.tensor_sub(out=b_sb[:], in0=beta2[:], in1=b_sb[:])
            h2x = big.tile([128, N], F16, tag="h2x")
            nc.vector.tensor_scalar(out=h2x[:], in0=x2x[:],
                                    scalar1=a_sb[:], scalar2=b_sb[:],
                                    op0=mybir.AluOpType.mult,
                                    op1=mybir.AluOpType.add)

            # ---- QKV projections (bf16, K=128) ----
            q2x = big.tile([128, N], F16, tag="q2x")
            k2x = big.tile([128, N], F16, tag="k2x")
            for j in range(N_NT):
                sl = slice(j * NT, (j + 1) * NT)
                qp = qk_ps.tile([128, 2 * NT], FP, tag="qk")
                nc.tensor.matmul(out=qp[:, 0:NT], lhsT=wq_st[:], rhs=h2x[:, sl])
                nc.tensor.matmul(out=qp[:, NT:2 * NT], lhsT=wk_st[:], rhs=h2x[:, sl])
                # q needs a bias add (VectorE); k is a plain copy (ScalarE)
                nc.vector.tensor_scalar_add(out=q2x[:, sl], in0=qp[:, 0:NT], scalar1=bq2[:])
                nc.scalar.activation(out=k2x[:, sl], in_=qp[:, NT:2 * NT], func=COPY)

            # vT_aug[p, mt, 0:64] = v[m = mt*128+p, c]; vT_aug[p, mt, 64] = 1
            vT = big.tile([128, N_MT, C + 1], F16, tag="vT")
            nc.vector.memset(vT[:, :, C:C + 1], 1.0)
            for mt in range(0, N_MT, 4):
                vp = av_ps.tile([128, NT], FP, tag="av")
                for j in range(4):
                    nc.tensor.matmul(out=vp[:, j * C:(j + 1) * C],
                                     lhsT=h2x[:, (mt + j) * MT:(mt + j + 1) * MT],
                                     rhs=wv_st[:])
                nc.scalar.activation(
                    out=vT[:, mt:mt + 4, 0:C],
                    in_=vp[:, 0:4 * C].rearrange("p (j c) -> p j c", j=4),
                    func=COPY)

            if debug:
                dh = big.tile([128, N], FP, tag="dbg_h_f32")
                dq = big.tile([128, N], FP, tag="dbgq")
                dk = big.tile([128, N], FP, tag="dbgk")
                dv = big.tile([128, N_MT * (C + 1)], FP, tag="dbgv")
                nc.vector.tensor_copy(out=dh[:], in_=h2x[:])
                nc.vector.tensor_copy(out=dq[:], in_=q2x[:])
                nc.vector.tensor_copy(out=dk[:], in_=k2x[:])
                nc.vector.tensor_copy(out=dv[:], in_=vT[:].rearrange("p a b -> p (a b)"))
                nc.sync.dma_start(out=dbg["dbg_h"][:], in_=dh[:])
                nc.sync.dma_start(out=dbg["dbg_q"][:], in_=dq[:])
                nc.sync.dma_start(out=dbg["dbg_k"][:], in_=dk[:])
                nc.sync.dma_start(out=dbg["dbg_vt"][:], in_=dv[:])

            # ---- attention (software-pipelined over n-tiles) ----
            e_tiles = {}

            # m-chunk grouping per n-tile: 10 groups of 3 + 1 of 2 so each
            # exp instruction covers [128, 1536] (amortizes ScalarE's
            # per-instruction overhead; 3 PSUM banks per group).
            GROUPS = [(i * 3, 3) for i in range(10)] + [(30, 2)]

            def emit_qk_group(nt, g, e):
                nsl = slice(nt * NT, (nt + 1) * NT)
                mt0, gsz = GROUPS[g]
                sp = qk_ps.tile([128, 3 * NT], FP, tag="qk")
                for j in range(gsz):
                    mt = mt0 + j
                    nc.tensor.matmul(out=sp[:, j * NT:(j + 1) * NT],
                                     lhsT=k2x[:, mt * MT:(mt + 1) * MT],
                                     rhs=q2x[:, nsl])
                nc.scalar.activation(out=e[:, mt0:mt0 + gsz, :],
                                     in_=sp[:, 0:gsz * NT],
                                     func=mybir.ActivationFunctionType.Exp)

            def emit_av_group(av, e, g):
                mt0, gsz = GROUPS[g]
                for j in range(gsz):
                    mt = mt0 + j
                    nc.tensor.matmul(
                        out=av[0:C + 1, :],
                        lhsT=vT[:, mt, :],
                        rhs=e[:, mt, :],
                        start=(mt == 0), stop=(mt == N_MT - 1),
                        skip_group_check=True)

            def emit_post(nt, av):
                nsl = slice(nt * NT, (nt + 1) * NT)
                # den (psum row 64) -> SBUF -> partition-broadcast via DMA
                # (DRAM bounce) -> fast approx reciprocal on 64 partitions.
                den_sb = small.tile([128, NT], FP, tag="den_sb")
                nc.vector.tensor_copy(out=den_sb[C:C + 1, :], in_=av[C:C + 1, :])
                if debug:
                    nc.sync.dma_start(out=dbg["dbg_den"][:, nsl], in_=den_sb[C:C + 1, :])
                den_dr = drampool.tile([1, NT], FP)
                nc.sync.dma_start(out=den_dr[:], in_=den_sb[C:C + 1, :])
                den_bc = outp.tile([C, NT], FP, tag="den_bc")
                nc.sync.dma_start(
                    out=den_bc[:],
                    in_=bass.AP(tensor=den_dr.tensor, offset=den_dr.offset,
                                ap=[[0, C]] + list(den_dr.ap[1:])))
                dbc = outp.tile([C, NT], FP, tag="dbc")
                scr = outp.tile([C, NT], FP, tag="dbc_scr")
                nc.vector.reciprocal_approx_accurate(out=dbc[:], in_=den_bc[:], scratch=scr[:])
                # unnormalized attention output -> SBUF (bf16) for proj matmul
                av_sb = outp.tile([C, NT], F16, tag="av_sb")
                nc.vector.tensor_copy(out=av_sb[:], in_=av[0:C, :])
                if debug:
                    dav = outp.tile([C, NT], FP, tag="dav")
                    nc.vector.tensor_copy(out=dav[:], in_=av[0:C, :])
                    nc.sync.dma_start(out=dbg["dbg_av"][:, nsl], in_=dav[:])
                # proj, then scale columns by 1/den, + bias' + residual
                pj_ps = post_ps.tile([128, 512], FP, tag="post")
                nc.tensor.matmul(out=pj_ps[0:C, :], lhsT=wpT[:], rhs=av_sb[:])
                o_sb = outp.tile([C, NT], FP, tag="o_sb")
                nc.vector.tensor_mul(out=o_sb[:], in0=pj_ps[0:C, :], in1=dbc[:])
                nc.vector.scalar_tensor_tensor(
                    out=o_sb[:], in0=o_sb[:], scalar=bpp[:], in1=x2x[0:C, nsl],
                    op0=mybir.AluOpType.add, op1=mybir.AluOpType.add)
                nc.sync.dma_start(out=out_d[:, nsl], in_=o_sb[:])

            for nt in range(N_NT + 1):
                e_cur = None
                if nt < N_NT:
                    e_cur = epool.tile([128, N_MT, NT], F16, tag="e")
                    e_tiles[nt] = e_cur
                if nt > 0:
                    av_cur = av_ps.tile([128, NT], FP, tag="av", name=f"av_{nt}")
                else:
                    av_cur = None
                for g in range(len(GROUPS)):
                    if e_cur is not None:
                        emit_qk_group(nt, g, e_cur)
                    if av_cur is not None:
                        emit_av_group(av_cur, e_tiles[nt - 1], g)
                if nt > 0:
                    e_tiles.pop(nt - 1)
                    emit_post(nt - 1, av_cur)

    nc.finalize()  # Bacc.finalize runs the wait-splitting legalization
    return nc


_cached = {}


def _install_trace_hook():
    """The agent image lacks antenv.axon_hooks, so run_bass_kernel_spmd's
    trace path degrades. Recreate the module + NTFF hook locally."""
    import sys, types
    import antenv
    if "antenv.axon_hooks" in sys.modules:
        return
    mod = types.ModuleType("antenv.axon_hooks")
    holder = {"hook": None}
    mod.set_axon_ntff_profile_hook = lambda h: holder.__setitem__("hook", h)
    mod.get_axon_ntff_profile_hook = lambda: holder["hook"]
    sys.modules["antenv.axon_hooks"] = mod
    antenv.axon_hooks = mod
    from trn_agent_boot.trn_boot import _ntff_profile_via_ctypes
    mod.set_axon_ntff_profile_hook(_ntff_profile_via_ctypes("/opt/axon/libaxon_pjrt.so"))
    import concourse.bass_utils as bu
    bu.upload_artifacts = lambda tmpdir: tmpdir


def make_consts(Wq, bq, Wk, Wv, bv, Wp, bp, gn_w, gn_b):
    f32 = lambda a: np.ascontiguousarray(np.asarray(a, np.float32))
    f16 = lambda a: np.ascontiguousarray(np.asarray(a, np.float32).astype(ml_dtypes.bfloat16))
    gmask = np.zeros((C, G), np.float32)
    gbcast2 = np.zeros((G, 128), np.float32)
    for g in range(G):
        gmask[g * 8:(g + 1) * 8, g] = 1.0 / 8.0
        gbcast2[g, g * 8:(g + 1) * 8] = 1.0
        gbcast2[g, C + g * 8:C + (g + 1) * 8] = 1.0
    WqT = np.asarray(Wq, np.float32).T
    WkT = np.asarray(Wk, np.float32).T
    WvT = np.asarray(Wv, np.float32).T
    return {
        "wq_st": f16(np.tile(WqT, (2, 2)) / 32.0),
        "wk_st": f16(np.tile(WkT, (2, 2)) / 2.0),
        "wv_st": f16(np.tile(WvT, (2, 1)) / 2.0),
        "bq2": f32(np.tile(np.asarray(bq, np.float32) / 16.0, 2))[:, None],
        "wpT": f16(np.asarray(Wp).T),
        "bpp": f32(np.asarray(bp) + np.asarray(Wp) @ np.asarray(bv))[:, None],
        "gamma2": f32(np.tile(np.asarray(gn_w, np.float32), 2))[:, None],
        "beta2": f32(np.tile(np.asarray(gn_b, np.float32), 2))[:, None],
        "gmask": gmask,
        "gbcast2": gbcast2,
    }


def kernel(x, gn_w, gn_b, Wq, bq, Wk, bk, Wv, bv, Wp, bp, _trace=False):
    x = np.ascontiguousarray(np.asarray(x, np.float32)).reshape(B, C, N)
    consts = make_consts(Wq, bq, Wk, Wv, bv, Wp, bp, gn_w, gn_b)

    if _trace:
        _install_trace_hook()

    if "nc" not in _cached:
        _cached["nc"] = build_program()
    nc = _cached["nc"]

    in_maps = [dict(consts, x=np.ascontiguousarray(x[i])) for i in range(B)]
    res = run_bass_kernel_spmd(nc, in_maps, core_ids=list(range(B)), trace=_trace)
    last_run_info["exec_time_ns"] = res.exec_time_ns
    last_run_info["mean_exec_time_ns"] = res.mean_exec_time_ns
    out = np.stack([res.results[i]["out"] for i in range(B)], axis=0)
    return out.reshape(B, C, H, W)


# revision 32
# speedup vs baseline: 1.1595x; 1.1595x over previous
"""AttentionBlock (GroupNorm -> 1x1-conv QKV -> softmax attention -> 1x1-conv proj
-> residual) for Trainium2, data-parallel over batch across 8 NeuronCores.

Shapes (hardcoded): x [B=8, C=64, H=64, W=64] fp32; N = H*W = 4096.
Each core processes one sample end-to-end; no cross-core communication.

Key Trainium facts that shape this kernel:
  - A matmul with contraction K<=64 streams at HALF rate (64-row tiling mode);
    K=128 streams 1 column/cycle. With C=64 channels, all hot matmuls are
    made K=128 by duplicating operands on both partition halves and halving
    the stacked weights (sum over 128 partitions of duplicated data = 2x).
  - fp32 matmuls run as two PE passes and their self-loading LDWEIGHTS only
    supports one sync wait; bf16 is one pass (and scores are O(1), so bf16
    keeps ~3 digits -> final error ~1e-4).
  - ScalarE exp runs at 1 elem/lane/cycle -> 16.7M exps/core ~ 115us is the
    roofline engine; everything else is arranged to hide under it.

Per-core pipeline:
  1. GroupNorm: per-channel bn_stats/bn_aggr on x2x[0:64] -> tiny mask
     matmuls reduce/broadcast the 8-channel groups -> one fused affine
     produces h2x [128, N] bf16 (h duplicated on both partition halves).
  2. q2x = (Wq h + bq)/16 and k2x = Wk h, both [128, N] bf16 duplicated
     (bk dropped: constant shift per softmax row). vT [N, C+1] bf16 with a
     ones column so the AV matmul also accumulates the softmax denominator.
  3. sT[m, n] tiles = k2x.T @ q2x (K=128), exp on ScalarE PSUM->SBUF (score
     range is ~[-3, 3]: no row-max subtraction needed), AV accumulates
     out[c, n] + den[n] over the 32 m-chunks.
  4. proj = Wp @ out_unnormalized, scaled by 1/den (column scaling commutes
     with the left matmul; reciprocal via a DMA partition-broadcast of den
     and the fast DVE approx reciprocal), + (bp + Wp bv) + residual x.

The nt loop is software-pipelined (scores/exp of tile nt interleaved with AV
of tile nt-1) so the PE stream stays dense and ScalarE never starves.
"""

import os
import numpy as np
import ml_dtypes

import concourse.bass as bass
import concourse.bacc as bacc
import concourse.mybir as mybir
from concourse.tile import TileContext
from concourse.bass_utils import run_bass_kernel_spmd

FP = mybir.dt.float32
F16 = mybir.dt.bfloat16
B, C, H, W = 8, 64, 64, 64
N = H * W          # 4096
G = 8              # groups
NT = 512           # n-tile (free dim of score tiles)
MT = 128           # m-tile (partition dim of score tiles)
N_NT = N // NT     # 8
N_MT = N // MT     # 32
NPAIR = N_MT // 2  # 16 score psum groups (2 m-chunks each) per n-tile
EPS = 1e-5
COPY = mybir.ActivationFunctionType.Copy

last_run_info = {}


def build_program(debug=False):
    # Bacc (not raw Bass): its finalize pipeline splits multi-sem waits.
    nc = bacc.Bacc()
    dbg = {}
    if debug:
        for nm, shp in [("dbg_h", [128, N]), ("dbg_q", [128, N]), ("dbg_k", [128, N]),
                        ("dbg_vt", [128, N_MT * (C + 1)]),
                        ("dbg_av", [C, N]), ("dbg_den", [1, N])]:
            dbg[nm] = nc.dram_tensor(nm, shp, FP, kind="ExternalOutput")

    x_d = nc.dram_tensor("x", [C, N], FP, kind="ExternalInput")
    wq_st_d = nc.dram_tensor("wq_st", [128, 128], F16, kind="ExternalInput")  # tile(Wq.T,(2,2))/32
    wk_st_d = nc.dram_tensor("wk_st", [128, 128], F16, kind="ExternalInput")  # tile(Wk.T,(2,2))/2
    wv_st_d = nc.dram_tensor("wv_st", [128, C], F16, kind="ExternalInput")    # tile(Wv.T,(2,1))/2
    bq2_d = nc.dram_tensor("bq2", [128, 1], FP, kind="ExternalInput")         # tile(bq,2)/16
    wpT_d = nc.dram_tensor("wpT", [C, C], F16, kind="ExternalInput")          # Wp.T
    bpp_d = nc.dram_tensor("bpp", [C, 1], FP, kind="ExternalInput")           # bp + Wp@bv
    gamma2_d = nc.dram_tensor("gamma2", [128, 1], FP, kind="ExternalInput")   # tile(gn_w,2)
    beta2_d = nc.dram_tensor("beta2", [128, 1], FP, kind="ExternalInput")     # tile(gn_b,2)
    gmask_d = nc.dram_tensor("gmask", [C, G], FP, kind="ExternalInput")       # 1/8 blocks
    gbcast2_d = nc.dram_tensor("gbcast2", [G, 128], FP, kind="ExternalInput") # 1 blocks x2
    out_d = nc.dram_tensor("out", [C, N], FP, kind="ExternalOutput")

    with TileContext(nc) as tc:
        with (
            tc.tile_pool(name="const", bufs=1) as const,
            tc.tile_pool(name="big", bufs=1) as big,
            tc.tile_pool(name="epool", bufs=2) as epool,
            tc.tile_pool(name="small", bufs=4) as small,
            tc.tile_pool(name="outp", bufs=3) as outp,
            tc.tile_pool(name="dram", bufs=2, space="DRAM") as drampool,
            tc.tile_pool(name="qk_ps", bufs=2, space="PSUM") as qk_ps,
            tc.tile_pool(name="av_ps", bufs=2, space="PSUM") as av_ps,
            tc.tile_pool(name="post_ps", bufs=2, space="PSUM") as post_ps,
        ):
            # ---- constant loads ----
            bq2 = const.tile([128, 1], FP, tag="bq2")
            bpp = const.tile([C, 1], FP, tag="bpp")
            gamma2 = const.tile([128, 1], FP, tag="gamma2")
            beta2 = const.tile([128, 1], FP, tag="beta2")
            for t, d in [(bq2, bq2_d), (bpp, bpp_d), (gamma2, gamma2_d), (beta2, beta2_d)]:
                nc.sync.dma_start(out=t[:], in_=d[:])
            # Matmul operands straight off DMA would need DMA+DVE waits, but
            # a matmul's LDWEIGHTS supports only one sync wait. Funnel
            # weights through a DVE copy so matmul deps collapse onto DVE.
            wq_st = const.tile([128, 128], F16, tag="wq_st")
            wk_st = const.tile([128, 128], F16, tag="wk_st")
            wv_st = const.tile([128, C], F16, tag="wv_st")
            wpT = const.tile([C, C], F16, tag="wpT")
            gmask = const.tile([C, G], FP, tag="gmask")
            gbcast2 = const.tile([G, 128], FP, tag="gbcast2")
            for t, d in [(wq_st, wq_st_d), (wk_st, wk_st_d), (wv_st, wv_st_d),
                         (wpT, wpT_d), (gmask, gmask_d), (gbcast2, gbcast2_d)]:
                stg = small.tile(list(t.shape), t.dtype, tag=f"stage_{t.shape[1]}_{t.dtype}")
                nc.sync.dma_start(out=stg[:], in_=d[:])
                nc.vector.tensor_copy(out=t[:], in_=stg[:])

            eps_sb = const.tile([128, 1], FP, tag="eps")
            nc.vector.memset(eps_sb[:], EPS)

            # ---- load x duplicated on both partition halves ----
            x2x = big.tile([128, N], FP, tag="x2x")
            for j in range(4):
                sl = slice(j * (N // 4), (j + 1) * (N // 4))
                nc.sync.dma_start(out=x2x[0:C, sl], in_=x_d[:, sl])
                nc.sync.dma_start(out=x2x[C:128, sl], in_=x_d[:, sl])

            # ---- GroupNorm ----
            stats = small.tile([C, N // 512, 6], FP, tag="gn_stats")
            for j in range(N // 512):
                nc.vector.bn_stats(out=stats[:, j, :], in_=x2x[0:C, j * 512:(j + 1) * 512])
            mv = small.tile([C, 2], FP, tag="gn_mv")
            nc.vector.bn_aggr(out=mv[:], in_=stats[:])
            # mm2 = [mean_c, mean_c^2 + var_c]
            mm2 = small.tile([C, 2], FP, tag="gn_mm2")
            nc.vector.tensor_copy(out=mm2[:, 0:1], in_=mv[:, 0:1])
            t0 = small.tile([C, 1], FP, tag="gn_t0")
            nc.vector.tensor_mul(out=t0[:], in0=mv[:, 0:1], in1=mv[:, 0:1])
            nc.vector.tensor_add(out=mm2[:, 1:2], in0=t0[:], in1=mv[:, 1:2])
            # group stats: [G, 2] = gmask.T @ mm2   (gmask holds 1/8)
            gstat_ps = post_ps.tile([128, 512], FP, tag="post")
            nc.tensor.matmul(out=gstat_ps[0:G, 0:2], lhsT=gmask[:], rhs=mm2[:])
            gstat = small.tile([G, 2], FP, tag="gn_gstat")
            nc.vector.tensor_copy(out=gstat[:], in_=gstat_ps[0:G, 0:2])
            # var_g = E[x^2]_g - mean_g^2 ; rstd = 1/sqrt(var+eps)
            vg = small.tile([G, 1], FP, tag="gn_vg")
            nc.vector.tensor_mul(out=vg[:], in0=gstat[:, 0:1], in1=gstat[:, 0:1])
            nc.vector.tensor_sub(out=vg[:], in0=gstat[:, 1:2], in1=vg[:])
            stdg = small.tile([G, 1], FP, tag="gn_stdg")
            nc.scalar.activation(out=stdg[:], in_=vg[:],
                                 func=mybir.ActivationFunctionType.Sqrt,
                                 bias=eps_sb[0:G, :])
            rhs2 = small.tile([G, 2], FP, tag="gn_rhs2")
            nc.vector.tensor_copy(out=rhs2[:, 0:1], in_=gstat[:, 0:1])
            nc.vector.reciprocal(out=rhs2[:, 1:2], in_=stdg[:])
            # broadcast to both channel copies: [128, 2] = gbcast2.T @ rhs2
            pstat_ps = post_ps.tile([128, 512], FP, tag="post")
            nc.tensor.matmul(out=pstat_ps[:, 0:2], lhsT=gbcast2[:], rhs=rhs2[:])
            a_sb = small.tile([128, 1], FP, tag="gn_a")
            b_sb = small.tile([128, 1], FP, tag="gn_b")
            nc.vector.tensor_mul(out=a_sb[:], in0=pstat_ps[:, 1:2], in1=gamma2[:])
            nc.vector.tensor_mul(out=b_sb[:], in0=pstat_ps[:, 0:1], in1=a_sb[:])
            nc.vector.tensor_sub(out=b_sb[:], in0=beta2[:], in1=b_sb[:])
            h2x = big.tile([128, N], F16, tag="h2x")
            nc.vector.tensor_scalar(out=h2x[:], in0=x2x[:],
                                    scalar1=a_sb[:], scalar2=b_sb[:],
                                    op0=mybir.AluOpType.mult,
                                    op1=mybir.AluOpType.add)

            # ---- QKV projections (bf16, K=128) ----
            q2x = big.tile([128, N], F16, tag="q2x")
            k2x = big.tile([128, N], F16, tag="k2x")
            for j in range(N_NT):
                sl = slice(j * NT, (j + 1) * NT)
                qp = qk_ps.tile([128, 2 * NT], FP, tag="qk")
                nc.tensor.matmul(out=qp[:, 0:NT], lhsT=wq_st[:], rhs=h2x[:, sl])
                nc.tensor.matmul(out=qp[:, NT:2 * NT], lhsT=wk_st[:], rhs=h2x[:, sl])
                # q needs a bias add (VectorE); k is a plain copy (ScalarE)
                nc.vector.tensor_scalar_add(out=q2x[:, sl], in0=qp[:, 0:NT], scalar1=bq2[:])
                nc.scalar.activation(out=k2x[:, sl], in_=qp[:, NT:2 * NT], func=COPY)

            # vT_aug[p, mt, 0:64] = v[m = mt*128+p, c]; vT_aug[p, mt, 64] = 1
            vT = big.tile([128, N_MT, C + 1], F16, tag="vT")
            nc.vector.memset(vT[:, :, C:C + 1], 1.0)
            for mt in range(0, N_MT, 4):
                vp = av_ps.tile([128, NT], FP, tag="av")
                for j in range(4):
                    nc.tensor.matmul(out=vp[:, j * C:(j + 1) * C],
                                     lhsT=h2x[:, (mt + j) * MT:(mt + j + 1) * MT],
                                     rhs=wv_st[:])
                nc.scalar.activation(
                    out=vT[:, mt:mt + 4, 0:C],
                    in_=vp[:, 0:4 * C].rearrange("p (j c) -> p j c", j=4),
                    func=COPY)

            if debug:
                dh = big.tile([128, N], FP, tag="dbg_h_f32")
                dq = big.tile([128, N], FP, tag="dbgq")
                dk = big.tile([128, N], FP, tag="dbgk")
                dv = big.tile([128, N_MT * (C + 1)], FP, tag="dbgv")
                nc.vector.tensor_copy(out=dh[:], in_=h2x[:])
                nc.vector.tensor_copy(out=dq[:], in_=q2x[:])
                nc.vector.tensor_copy(out=dk[:], in_=k2x[:])
                nc.vector.tensor_copy(out=dv[:], in_=vT[:].rearrange("p a b -> p (a b)"))
                nc.sync.dma_start(out=dbg["dbg_h"][:], in_=dh[:])
                nc.sync.dma_start(out=dbg["dbg_q"][:], in_=dq[:])
                nc.sync.dma_start(out=dbg["dbg_k"][:], in_=dk[:])
                nc.sync.dma_start(out=dbg["dbg_vt"][:], in_=dv[:])

            # ---- attention (software-pipelined over n-tiles) ----
            e_tiles = {}

            # m-chunk grouping per n-tile: 10 groups of 3 + 1 of 2 so each
            # exp instruction covers [128, 1536] (amortizes ScalarE's
            # per-instruction overhead; 3 PSUM banks per group).
            GROUPS = [(i * 2, 2) for i in range(16)]

            def emit_qk_group(nt, g, e):
                nsl = slice(nt * NT, (nt + 1) * NT)
                mt0, gsz = GROUPS[g]
                sp = qk_ps.tile([128, 2 * NT], FP, tag="qk")
                for j in range(gsz):
                    mt = mt0 + j
                    nc.tensor.matmul(out=sp[:, j * NT:(j + 1) * NT],
                                     lhsT=k2x[:, mt * MT:(mt + 1) * MT],
                                     rhs=q2x[:, nsl])
                nc.scalar.activation(out=e[:, mt0:mt0 + gsz, :],
                                     in_=sp[:, 0:gsz * NT],
                                     func=mybir.ActivationFunctionType.Exp)

            def emit_av_group(av, e, g):
                mt0, gsz = GROUPS[g]
                for j in range(gsz):
                    mt = mt0 + j
                    nc.tensor.matmul(
                        out=av[0:C + 1, :],
                        lhsT=vT[:, mt, :],
                        rhs=e[:, mt, :],
                        start=(mt == 0), stop=(mt == N_MT - 1),
                        skip_group_check=True)

            def emit_post(nt, av):
                nsl = slice(nt * NT, (nt + 1) * NT)
                # den (psum row 64) -> SBUF -> partition-broadcast via DMA
                # (DRAM bounce) -> fast approx reciprocal on 64 partitions.
                den_sb = small.tile([128, NT], FP, tag="den_sb")
                nc.vector.tensor_copy(out=den_sb[C:C + 1, :], in_=av[C:C + 1, :])
                if debug:
                    nc.sync.dma_start(out=dbg["dbg_den"][:, nsl], in_=den_sb[C:C + 1, :])
                den_dr = drampool.tile([1, NT], FP)
                nc.sync.dma_start(out=den_dr[:], in_=den_sb[C:C + 1, :])
                den_bc = outp.tile([C, NT], FP, tag="den_bc")
                nc.sync.dma_start(
                    out=den_bc[:],
                    in_=bass.AP(tensor=den_dr.tensor, offset=den_dr.offset,
                                ap=[[0, C]] + list(den_dr.ap[1:])))
                dbc = outp.tile([C, NT], FP, tag="dbc")
                scr = outp.tile([C, NT], FP, tag="dbc_scr")
                nc.vector.reciprocal_approx_accurate(out=dbc[:], in_=den_bc[:], scratch=scr[:])
                # unnormalized attention output -> SBUF (bf16) for proj matmul
                av_sb = outp.tile([C, NT], F16, tag="av_sb")
                nc.vector.tensor_copy(out=av_sb[:], in_=av[0:C, :])
                if debug:
                    dav = outp.tile([C, NT], FP, tag="dav")
                    nc.vector.tensor_copy(out=dav[:], in_=av[0:C, :])
                    nc.sync.dma_start(out=dbg["dbg_av"][:, nsl], in_=dav[:])
                # proj, then scale columns by 1/den, + bias' + residual
                pj_ps = post_ps.tile([128, 512], FP, tag="post")
                nc.tensor.matmul(out=pj_ps[0:C, :], lhsT=wpT[:], rhs=av_sb[:])
                o_sb = outp.tile([C, NT], FP, tag="o_sb")
                nc.vector.tensor_mul(out=o_sb[:], in0=pj_ps[0:C, :], in1=dbc[:])
                nc.vector.scalar_tensor_tensor(
                    out=o_sb[:], in0=o_sb[:], scalar=bpp[:], in1=x2x[0:C, nsl],
                    op0=mybir.AluOpType.add, op1=mybir.AluOpType.add)
                nc.sync.dma_start(out=out_d[:, nsl], in_=o_sb[:])

            for nt in range(N_NT + 1):
                e_cur = None
                if nt < N_NT:
                    e_cur = epool.tile([128, N_MT, NT], F16, tag="e")
                    e_tiles[nt] = e_cur
                if nt > 0:
                    av_cur = av_ps.tile([128, NT], FP, tag="av", name=f"av_{nt}")
                else:
                    av_cur = None
                for g in range(len(GROUPS)):
                    if e_cur is not None:
                        emit_qk_group(nt, g, e_cur)
                    if av_cur is not None:
                        emit_av_group(av_cur, e_tiles[nt - 1], g)
                if nt > 0:
                    e_tiles.pop(nt - 1)
                    emit_post(nt - 1, av_cur)

    nc.finalize()  # Bacc.finalize runs the wait-splitting legalization
    return nc


_cached = {}


def _install_trace_hook():
    """The agent image lacks antenv.axon_hooks, so run_bass_kernel_spmd's
    trace path degrades. Recreate the module + NTFF hook locally."""
    import sys, types
    import antenv
    if "antenv.axon_hooks" in sys.modules:
        return
    mod = types.ModuleType("antenv.axon_hooks")
    holder = {"hook": None}
    mod.set_axon_ntff_profile_hook = lambda h: holder.__setitem__("hook", h)
    mod.get_axon_ntff_profile_hook = lambda: holder["hook"]
    sys.modules["antenv.axon_hooks"] = mod
    antenv.axon_hooks = mod
    from trn_agent_boot.trn_boot import _ntff_profile_via_ctypes
    mod.set_axon_ntff_profile_hook(_ntff_profile_via_ctypes("/opt/axon/libaxon_pjrt.so"))
    import concourse.bass_utils as bu
    bu.upload_artifacts = lambda tmpdir: tmpdir


def make_consts(Wq, bq, Wk, Wv, bv, Wp, bp, gn_w, gn_b):
    f32 = lambda a: np.ascontiguousarray(np.asarray(a, np.float32))
    f16 = lambda a: np.ascontiguousarray(np.asarray(a, np.float32).astype(ml_dtypes.bfloat16))
    gmask = np.zeros((C, G), np.float32)
    gbcast2 = np.zeros((G, 128), np.float32)
    for g in range(G):
        gmask[g * 8:(g + 1) * 8, g] = 1.0 / 8.0
        gbcast2[g, g * 8:(g + 1) * 8] = 1.0
        gbcast2[g, C + g * 8:C + (g + 1) * 8] = 1.0
    WqT = np.asarray(Wq, np.float32).T
    WkT = np.asarray(Wk, np.float32).T
    WvT = np.asarray(Wv, np.float32).T
    return {
        "wq_st": f16(np.tile(WqT, (2, 2)) / 32.0),
        "wk_st": f16(np.tile(WkT, (2, 2)) / 2.0),
        "wv_st": f16(np.tile(WvT, (2, 1)) / 2.0),
        "bq2": f32(np.tile(np.asarray(bq, np.float32) / 16.0, 2))[:, None],
        "wpT": f16(np.asarray(Wp).T),
        "bpp": f32(np.asarray(bp) + np.asarray(Wp) @ np.asarray(bv))[:, None],
        "gamma2": f32(np.tile(np.asarray(gn_w, np.float32), 2))[:, None],
        "beta2": f32(np.tile(np.asarray(gn_b, np.float32), 2))[:, None],
        "gmask": gmask,
        "gbcast2": gbcast2,
    }


def kernel(x, gn_w, gn_b, Wq, bq, Wk, bk, Wv, bv, Wp, bp, _trace=False):
    x = np.ascontiguousarray(np.asarray(x, np.float32)).reshape(B, C, N)
    consts = make_consts(Wq, bq, Wk, Wv, bv, Wp, bp, gn_w, gn_b)

    if _trace:
        _install_trace_hook()

    if "nc" not in _cached:
        _cached["nc"] = build_program()
    nc = _cached["nc"]

    in_maps = [dict(consts, x=np.ascontiguousarray(x[i])) for i in range(B)]
    res = run_bass_kernel_spmd(nc, in_maps, core_ids=list(range(B)), trace=_trace)
    last_run_info["exec_time_ns"] = res.exec_time_ns
    last_run_info["mean_exec_time_ns"] = res.mean_exec_time_ns
    out = np.stack([res.results[i]["out"] for i in range(B)], axis=0)
    return out.reshape(B, C, H, W)
